# revision 26
# baseline (speedup 1.0000x reference)
"""Single-head causal attention (B=4, S=4096, E=1024, H=64) on 8 Trainium2 cores.

Sharding: 8 cores = 4 batches x 2 query-range variants (M=3072 balances PE):
  - cores 0..3 (A): batch = core,   queries [0, M),  kv [0, M)
  - cores 4..7 (B): batch = core-4, queries [M, S),  kv [0, S)

The kernel minimizes per-execution DMA descriptor generation and
instruction-stream cost: X is loaded in 512-row blocks with 16KB-contiguous
runs per partition (partition p holds rows 4p..4p+3; 128 descriptors each),
all constants arrive pre-packed in one [128, 1794] tensor (two DMAs: the
small idt/tri/bias columns first so transposes can start immediately), and
the output is stored once per core with row-grouped 1KB runs. No GpSimd:
causal masking multiplies a precomputed 128x128 upper triangle on diagonal
tiles (DVE) plus memsets for fully-masked sub-tiles.

Per-core pipeline (software-pipelined emission: projection items of block
b+1 round-robin with attention pairs of block b so the PE-bound projection
phase overlaps the ACT(exp)-bound attention phase):
  X block DMA -> PE-transpose to X^T (token-interleaved layout fixed by
  strided ACT/DVE copies) -> packed projection chains ([Wk|Wv] -> psum
  partitions 0:64/64:128, [Wq] -> psum 0:64, so K^T and Q^T share base
  partition 0 and need no shift-DMA; one activation applies both biases) ->
  attention per 512-token q-tile: scores^T k-tile pairs on PE, exp on ACT
  (scale=1/8 fused, no max-subtraction needed), triangle mask, PV+rowsum
  via one [V|1] matmul chain into a persistent PSUM accumulator.
Variant B loads its own rows first (superblock order 3,0,1,2), projects Q
once, then flash-accumulates PV over the whole kv sweep in two persistent
PSUM banks. Per-tile epilogue: 4 class transposes (token = 4p+g), one
strided reciprocal, one broadcast multiply; single grouped store at end.
"""

import numpy as np
import concourse.bass as bass
import concourse.mybir as mybir
import concourse.tile as tile
from concourse import bacc
from concourse.bass_utils import run_bass_kernel_spmd

F32R = mybir.dt.float32r
F32 = mybir.dt.float32
EXP = mybir.ActivationFunctionType.Exp
IDENT = mybir.ActivationFunctionType.Identity
MULT = mybir.AluOpType.mult

B, S, E, H = 4, 4096, 1024, 64
M_SPLIT = 3072
SB = 1024           # superblock rows (one X DMA)
BLK = 512           # projection block == q-tile width
KT = 128            # k-tile width
SCALE = 0.125       # 1/sqrt(64)
NCONST = 1794       # wkv 1024 | wq 512 | idt 128 | tri 128 | bkv 1 | bq 1


def build_program(s=S, e=E, m=M_SPLIT, time_reps=1):
    ec = e // 128
    nc = bacc.Bacc("TRN2", target_bir_lowering=False, debug=False, num_devices=8)

    X = nc.dram_tensor("X", [s, e], F32R, kind="ExternalInput")
    CONST = nc.dram_tensor("CONST", [128, NCONST], F32R, kind="ExternalInput")
    OUT = nc.dram_tensor("OUT", [s, H], F32, kind="ExternalOutput")

    with tile.TileContext(nc) as tc:
        from contextlib import ExitStack
        with ExitStack() as ctx:
            constp = ctx.enter_context(tc.tile_pool(name="constp", bufs=1))
            xnp = ctx.enter_context(tc.tile_pool(name="xnp", bufs=2))
            xtp = ctx.enter_context(tc.tile_pool(name="xtp", bufs=2))
            kvp = ctx.enter_context(tc.tile_pool(name="kvp", bufs=s // BLK))
            vnp = ctx.enter_context(tc.tile_pool(name="vnp", bufs=s // BLK))
            qp = ctx.enter_context(tc.tile_pool(name="qp", bufs=2))
            ptp = ctx.enter_context(tc.tile_pool(name="ptp", bufs=4))
            osbp = ctx.enter_context(tc.tile_pool(name="osbp", bufs=1))
            outp = ctx.enter_context(tc.tile_pool(name="outp", bufs=1))
            recp = ctx.enter_context(tc.tile_pool(name="recp", bufs=2))
            # PSUM: ps_a 2x[128,1024]f32 (4 banks) + kv 1 + q 1 + o 2 = 8
            ps_a = ctx.enter_context(tc.tile_pool(name="ps_a", bufs=2, space="PSUM"))
            ps_kv = ctx.enter_context(tc.tile_pool(name="ps_kv", bufs=1, space="PSUM"))
            ps_q = ctx.enter_context(tc.tile_pool(name="ps_q", bufs=1, space="PSUM"))
            ps_o = ctx.enter_context(tc.tile_pool(name="ps_o", bufs=2, space="PSUM"))

            def emit(q_lo, q_hi, kv_hi, sb_order):
                nqt = (q_hi - q_lo) // BLK
                G = (q_hi - q_lo) // 128

                C = constp.tile([128, NCONST], F32R, tag="const")
                # idt/tri/biases first (tiny) so transposes start early;
                # weight columns follow, overlapped with the first X load
                nc.scalar.dma_start(C[:, 1536:NCONST], CONST.ap()[:, 1536:NCONST])
                nc.scalar.dma_start(C[:, 0:1536], CONST.ap()[:, 0:1536])
                wkv = C[:, 0:1024]
                wq = C[:, 1024:1536]
                idt = C[:, 1536:1664]
                tri = C[:, 1664:1792]
                bkv = C[:, 1792:1793].bitcast(F32)
                bq = C[0:64, 1793:1794].bitcast(F32)

                kv = {}    # per block: [128, 512] (K^T 0:64 | V^T 64:128)
                vn = {}    # per block: [128, 4*66] natural [V|1]
                q2 = {}    # per q-tile: [64, 512]
                osb = osbp.tile([66, q_hi - q_lo], F32R, tag="osb")
                outsb = outp.tile([128, G * H], F32, tag="outsb")
                o_ps = {}
                o_seen = {}   # per tile: kt count emitted (for start/stop)

                def pair_thunk(t, pair, last_flags):
                    """Thunk emitting one score+exp+mask+PV pair for tile t."""
                    q0 = q_lo + t * BLK

                    def run():
                        if t not in o_ps:
                            o_ps[t] = ps_o.tile([66, BLK], F32, tag="o",
                                                name=f"o{t}")
                        st = ps_a.tile([128, 2 * BLK], F32, tag="st")
                        for hf, kt in enumerate(pair):
                            nc.tensor.matmul(
                                st[:, hf * BLK:(hf + 1) * BLK],
                                kv[kt // 4][0:64, (kt % 4) * KT:(kt % 4 + 1) * KT],
                                q2[t][:],
                                start=True, stop=True)
                        pt = ptp.tile([128, 2 * BLK], F32R, tag="pt")
                        nc.scalar.activation(pt[:], st[:], EXP, scale=SCALE)
                        for hf, kt in enumerate(pair):
                            # causal mask: sub s covers q [q0+128s, +128);
                            # D = q0 + 128*(s - kt): 0 -> triangle, <0 -> zero
                            nz = sum(1 for sB in range(4)
                                     if q0 + 128 * (sB - kt) < 0)
                            if nz:
                                nc.vector.memset(
                                    pt[:, hf * BLK:hf * BLK + nz * 128]
                                    .bitcast(F32), 0.0)
                            for sB in range(4):
                                if q0 + 128 * (sB - kt) == 0:
                                    col = hf * BLK + sB * 128
                                    nc.vector.tensor_tensor(
                                        pt[:, col:col + 128],
                                        pt[:, col:col + 128], tri, MULT)
                        for hf, kt in enumerate(pair):
                            nc.tensor.matmul(
                                o_ps[t][:],
                                vn[kt // 4][:, (kt % 4) * 66:(kt % 4) * 66 + 66],
                                pt[:, hf * BLK:(hf + 1) * BLK],
                                start=(o_seen[t] == 0),
                                stop=last_flags[hf])
                            o_seen[t] += 1
                    return run

                def att_pair_thunks(t, kts):
                    """Pair thunks for k-tiles kts of tile t (+finish when done)."""
                    total = (q_lo + t * BLK + BLK) // KT
                    seen0 = o_seen.setdefault(t, 0)
                    ths = []
                    for pr in range(len(kts) // 2):
                        pair = kts[2 * pr:2 * pr + 2]
                        last = [seen0 + 2 * pr + 1 == total,
                                seen0 + 2 * pr + 2 == total]
                        ths.append(pair_thunk(t, pair, last))
                    if seen0 + len(kts) == total:
                        ths.append(lambda t=t: finish_tile(t))
                    return ths

                def finish_tile(t):
                    # copy PSUM->SBUF, then per-tile epilogue: 4 class
                    # transposes (token = 4p+g within this 512-tile), one
                    # strided reciprocal, one broadcast multiply.
                    nc.vector.tensor_copy(osb[:, t * BLK:(t + 1) * BLK], o_ps[t][:])
                    on_t = ps_a.tile([128, 2 * BLK], F32, tag="st")
                    for g in range(4):
                        nc.tensor.transpose(
                            on_t[:, g * 128:g * 128 + 66].bitcast(F32R),
                            osb[:, t * BLK:(t + 1) * BLK]
                            .rearrange("h (j g) -> h g j", g=4)[:, g, :],
                            idt[0:66, 0:66])
                    rec = recp.tile([128, 4], F32, tag="rec")
                    nc.vector.reciprocal(
                        rec[:],
                        on_t[:].rearrange("p (s c) -> p s c", c=128)[:, 0:4, 64:65])
                    nc.vector.tensor_tensor(
                        outsb[:, t * 4 * H:(t + 1) * 4 * H]
                        .rearrange("p (g h) -> p g h", g=4),
                        on_t[:].rearrange("p (s c) -> p s c", c=128)[:, 0:4, 0:64],
                        rec[:, :, None].to_broadcast((128, 4, H)),
                        MULT)

                def proj_thunks(blk):
                    """Projection work items for one 512-row block."""
                    box = {}

                    def dma_th():
                        xn = box["xn"] = xnp.tile([128, 4 * e], F32R, tag="xn", name="xn")
                        nc.sync.dma_start(
                            xn[:].rearrange("p (r e) -> p r e", r=4),
                            X.ap()[blk * BLK:(blk + 1) * BLK, :]
                            .rearrange("(p r) e -> p r e", r=4))
                        box["xt"] = xtp.tile([128, ec * BLK], F32R, tag="xt", name="xt")

                    def tr_th(cp):
                        def run():
                            xn, xt = box["xn"], box["xt"]
                            tr = ps_a.tile([128, 2 * BLK], F32R, tag="st")
                            for c2 in range(2):
                                ch = 2 * cp + c2
                                for r in range(4):
                                    nc.tensor.transpose(
                                        tr[:, c2 * BLK + r * 128:
                                           c2 * BLK + (r + 1) * 128],
                                        xn[:, r * e + ch * 128:
                                           r * e + (ch + 1) * 128],
                                        idt)
                            dst = (xt[:, 2 * cp * BLK:(2 * cp + 2) * BLK]
                                   .rearrange("p (c j r) -> p c r j", c=2, r=4))
                            src = tr[:].rearrange("p (c r j) -> p c r j",
                                                  c=2, j=128)
                            if cp % 2 == 0:
                                nc.scalar.activation(dst, src, IDENT)
                            else:
                                nc.vector.tensor_copy(dst, src)
                        return run

                    def kv_th():
                        xt = box["xt"]
                        pkv = ps_kv.tile([128, BLK], F32, tag="kv")
                        for ch in range(ec):
                            nc.tensor.matmul(
                                pkv[:], wkv[:, ch * 128:(ch + 1) * 128],
                                xt[:, ch * BLK:(ch + 1) * BLK],
                                start=(ch == 0), stop=(ch == ec - 1))
                        kv[blk] = kvp.tile([128, BLK], F32R, tag="kv",
                                           name=f"kv{blk}")
                        nc.scalar.activation(kv[blk][:], pkv[:], IDENT, bias=bkv)

                    def q_th():
                        xt = box["xt"]
                        pq = ps_q.tile([64, BLK], F32, tag="q")
                        for ch in range(ec):
                            nc.tensor.matmul(
                                pq[:], wq[:, ch * 64:(ch + 1) * 64],
                                xt[:, ch * BLK:(ch + 1) * BLK],
                                start=(ch == 0), stop=(ch == ec - 1))
                        t = (blk * BLK - q_lo) // BLK
                        q2[t] = qp.tile([64, BLK], F32R, tag="q2",
                                        name=f"q2_{t}")
                        nc.scalar.activation(q2[t][:], pq[:], IDENT, bias=bq)

                    def vn_th():
                        vn[blk] = vnp.tile([128, 4 * 66], F32R, tag="vn",
                                           name=f"vn{blk}")
                        nc.vector.memset(vn[blk][:].bitcast(F32), 1.0)
                        pvn = ps_kv.tile([128, BLK], F32R, tag="kv")
                        for g4 in range(4):
                            nc.tensor.transpose(
                                pvn[:, g4 * 64:(g4 + 1) * 64],
                                kv[blk][64:128, g4 * 128:(g4 + 1) * 128],
                                idt[64:128, 64:128])
                        nc.vector.tensor_copy(
                            vn[blk][:].rearrange("p (r c) -> p r c", c=66)
                            [:, :, 0:64],
                            pvn[:, 0:256].rearrange("p (r c) -> p r c", c=64))

                    ths = [dma_th] + [tr_th(cp) for cp in range(ec // 2)] \
                        + [kv_th]
                    if q_lo <= blk * BLK < q_hi:
                        ths.append(q_th)
                    ths.append(vn_th)
                    return ths

                def att_thunks(blk):
                    """Attention pairs unlocked by block blk's projections."""
                    if q_lo == 0:
                        t = blk
                        if t < nqt:
                            return att_pair_thunks(t, list(range(4 * (blk + 1))))
                        return []
                    ths = []
                    for t in range(nqt):
                        if t not in q2:
                            continue
                        q0 = q_lo + t * BLK
                        nkt = (q0 + BLK) // KT
                        kts = [k for k in range(4 * blk, 4 * blk + 4) if k < nkt]
                        if blk == q0 // BLK and blk % 2 == 1:
                            kts = list(range(4 * (blk - 1), 4 * blk)) + kts
                        if kts:
                            ths.extend(att_pair_thunks(t, kts))
                    return ths

                block_order = [b for sb in sb_order for b in (2 * sb, 2 * sb + 1)]
                pend = []
                for blk in block_order:
                    items = proj_thunks(blk)
                    # round-robin merge: proj items interleave with the
                    # previous block's attention pairs so PE (proj) and
                    # ACT (exp) phases overlap
                    while items or pend:
                        if items:
                            items.pop(0)()
                        if pend:
                            pend.pop(0)()
                    pend = att_thunks(blk)
                while pend:
                    pend.pop(0)()
                # single grouped store: row = q_lo + t*512 + 4p + g
                nc.scalar.dma_start(
                    OUT.ap()[q_lo:q_hi, :].rearrange("(t p g) h -> p t g h", g=4,
                                                     p=128),
                    outsb[:].rearrange("p (t g h) -> p t g h", g=4, h=H))

            def emit_maybe_looped(q_lo, q_hi, kv_hi, sb_order):
                if time_reps == 1:
                    emit(q_lo, q_hi, kv_hi, sb_order)
                elif time_reps < 0:
                    for _ in range(-time_reps):   # python-unrolled (probe)
                        emit(q_lo, q_hi, kv_hi, sb_order)
                else:
                    with tc.For_i(0, time_reps) as _i:
                        emit(q_lo, q_hi, kv_hi, sb_order)

            pid = nc.partition_id()
            with tc.If(pid < 4) as cmp:
                emit_maybe_looped(0, m, m, list(range(m // SB)))
            with cmp.Else():
                emit_maybe_looped(m, s, s, [m // SB] + list(range(m // SB)))

    nc.compile()
    return nc


def host_pack(Wk, bk, Wq, bq, Wv, bv):
    """Pack all constants into one [128, NCONST] f32 tensor."""
    Wk, Wq, Wv = (np.asarray(a, dtype=np.float32) for a in (Wk, Wq, Wv))
    bk, bq, bv = (np.asarray(a, dtype=np.float32) for a in (bk, bq, bv))
    C = np.zeros((128, NCONST), dtype=np.float32)
    for c in range(8):
        rows = slice(c * 128, (c + 1) * 128)
        C[:, c * 128:c * 128 + 64] = Wk[rows]
        C[:, c * 128 + 64:c * 128 + 128] = Wv[rows]
        C[:, 1024 + c * 64:1024 + (c + 1) * 64] = Wq[rows]
    C[:, 1536:1664] = np.eye(128, dtype=np.float32)
    C[:, 1664:1792] = np.triu(np.ones((128, 128), dtype=np.float32))
    C[0:64, 1792] = bk
    C[64:128, 1792] = bv
    C[0:64, 1793] = bq
    return C


_prog_cache = {}


def _get_program():
    if "nc" not in _prog_cache:
        _prog_cache["nc"] = build_program()
    return _prog_cache["nc"]


def kernel(X, Wk, bk, Wq, bq, Wv, bv):
    X = np.asarray(X, dtype=np.float32)
    C = host_pack(Wk, bk, Wq, bq, Wv, bv)
    nc = _get_program()
    in_maps = []
    for c in range(8):
        b = c % 4
        in_maps.append({"X": np.ascontiguousarray(X[b]), "CONST": C})
    res = run_bass_kernel_spmd(nc, in_maps, core_ids=list(range(8)))
    out = np.empty((B, S, H), dtype=np.float32)
    for b in range(4):
        out[b, :M_SPLIT] = res.results[b]["OUT"][:M_SPLIT]
        out[b, M_SPLIT:] = res.results[4 + b]["OUT"][M_SPLIT:]
    return out
